# revision 29
# baseline (speedup 1.0000x reference)
"""Multi-head attention (b=2, p=16, n=512, d=512, h=8, dh=64) on 8 TRN2 cores.

Data-parallel over the 32 (b,p) sequences: 4 sequences per core, no
collectives.  Per-core dataflow (everything "T" = feature-on-partition):

  xT  (d,n)  --W_qkv stationary-->  qT,kT (e,n)   [e-tile = 2 heads]
  xT chunks stationary, W_v moving ->  v natural (n,e)  -> vaug (j,h,65)
  dotsT[j,i] = kT_h.T-slice @ qT_h   (K=64, heads A/B at rows 0:64/64:128)
  expT = exp(scale * dotsT)          (ScalarE, PSUM->SBUF, bf16 out)
  od[0:65] (+ sums row 64) = vaug_h.T @ expT_h   (M=65, ones col -> sums)
  evac: ONE wide CAST od[0:65] -> E (bf16), cheap 4x-mode SBUF re-copies
    E[0:64] -> oT halves; sums row E[64] is SBUF so DMA-able.
  softmax denom: sums -> DRAM -> [64,32] batch -> reciprocal (DVE) ->
    DRAM pair-major reshuffle -> R = P2.T @ rec_pair (PE broadcast) ->
    oT *= R (DVE, in place)
  yT = W_out.T @ oT + b              (bias on ScalarE, bf16 out, per-dt DMA)

Cross-sequence software pipeline: QKV(s+1) + norm/proj(s-1) are
interleaved as filler into the attention pairs of sequence s so the
TensorEngine never starves.  Host transposes x into xT and casts the
bf16 yT output back to f32 y.
"""

import os
import sys

import numpy as np

for _p in ("/opt/trn_rl_repo", "/root/.axon_site/_ro/trn_rl_repo"):
    if os.path.isdir(_p) and _p not in sys.path:
        sys.path.insert(0, _p)

import concourse.bass as bass  # noqa: E402
import concourse.mybir as mybir  # noqa: E402
from concourse import bacc  # noqa: E402
from concourse.tile import TileContext  # noqa: E402

F32 = mybir.dt.float32
BF16 = mybir.dt.bfloat16

N_CORES = 8
SEQ_PER_CORE = 4  # (b*p)=32 sequences / 8 cores
N = 512  # tokens per sequence
D = 512  # model dim
HEADS = 8
DH = 64
SCALE = DH**-0.5
NT = N // 128  # 4 token tiles
DT = D // 128  # 4 dim tiles

EXP_F = mybir.ActivationFunctionType.Exp
COPY_F = mybir.ActivationFunctionType.Copy
MULT = mybir.AluOpType.mult


def build_nc():
    """Build the per-core SPMD Bass program (same program on all 8 cores)."""
    nc = bacc.Bacc("TRN2", target_bir_lowering=False)

    xT = nc.declare_dram_parameter(
        "xT", [SEQ_PER_CORE, DT, 128, N], BF16, isOutput=False
    )
    wqkv = nc.declare_dram_parameter("wqkv", [DT, 128, 3 * D], BF16, isOutput=False)
    wout = nc.declare_dram_parameter("wout", [DT, 128, D], BF16, isOutput=False)
    bout = nc.declare_dram_parameter("bout", [D], F32, isOutput=False)
    p2d = nc.declare_dram_parameter("p2d", [2, 128], BF16, isOutput=False)
    out = nc.declare_dram_parameter(
        "out", [SEQ_PER_CORE, DT, 128, N], BF16, isOutput=True
    )

    with TileContext(nc) as tc:
        with (
            tc.tile_pool(name="consts", bufs=1) as cpool,
            tc.tile_pool(name="xin", bufs=2) as xpool,
            tc.tile_pool(name="qk", bufs=2) as qkpool,
            tc.tile_pool(name="vaug", bufs=2) as vpool,
            tc.tile_pool(name="expt", bufs=3) as epool,
            tc.tile_pool(name="ot", bufs=2) as opool,
            tc.tile_pool(name="evac", bufs=2) as Epool,
            tc.tile_pool(name="small", bufs=2) as spool,
            tc.tile_pool(name="yout", bufs=2) as ypool,
            tc.tile_pool(name="psq", bufs=2, space="PSUM") as psq,
            tc.tile_pool(name="psd", bufs=1, space="PSUM") as psd,
            tc.tile_pool(name="pso", bufs=1, space="PSUM") as pso,
        ):
            # ---- constants: DMA issue order = startup critical path -------
            # (seq-0 x first, then q/k weight halves, then v/out weights)
            wq_sb = cpool.tile([128, DT, 3 * D], BF16, tag="wq")
            wo_sb = cpool.tile([128, DT, D], BF16, tag="wo")

            def const_dmas():
                # q weights, then k, then v; W_out last (tail-only use)
                for lo in (0, 512, 1024):
                    for dt in range(DT):
                        nc.sync.dma_start(
                            wq_sb[:, dt, lo : lo + 512],
                            wqkv[dt][:, lo : lo + 512],
                        )
                for dt in range(DT):
                    nc.sync.dma_start(wo_sb[:, dt, :], wout[dt])

            b_sb = cpool.tile([128, DT], F32, tag="b")
            nc.sync.dma_start(b_sb[:], bout.rearrange("(t p) -> p t", p=128))
            p2 = cpool.tile([2, 128], BF16, tag="p2")
            nc.sync.dma_start(p2[:], p2d[:])

            seq_tiles = {}
            seq_oT = {}
            seq_rec2 = {}
            seq_scr = {}

            def qkv_alloc(s):
                """Allocate per-seq tiles + start the xT DMAs (one per dt)."""
                xt = xpool.tile([128, DT, N], BF16, tag="x")
                for dt in range(DT):
                    nc.sync.dma_start(xt[:, dt, :], xT[s, dt])
                q_sb = qkpool.tile([128, DT, N], BF16, tag="q")
                k_sb = qkpool.tile([128, DT, N], BF16, tag="k")
                vaug = vpool.tile([128, NT, HEADS, DH + 1], BF16, tag="v")
                nc.vector.memset(vaug[:, :, :, DH : DH + 1], 1.0)
                seq_tiles[s] = (xt, q_sb, k_sb, vaug)

            def qkv_etile(s, et):
                """One QKV output tile (et 0..7 = qT/kT e-tiles, 8..11 = v
                natural n-tiles): 4 accumulating matmuls + evacuation."""
                xt, q_sb, k_sb, vaug = seq_tiles[s]
                ps = psq.tile([128, 512], F32, tag="ps")
                if et < 8:
                    for dt in range(DT):
                        nc.tensor.matmul(
                            ps[:],
                            lhsT=wq_sb[:, dt, et * 128 : (et + 1) * 128],
                            rhs=xt[:, dt, :],
                            start=(dt == 0),
                            stop=(dt == DT - 1),
                        )
                    if et < 4:
                        # q evac on ScalarE to balance engine load
                        nc.scalar.activation(q_sb[:, et, :], ps[:], COPY_F)
                    else:
                        nc.vector.tensor_copy(k_sb[:, et - 4, :], ps[:])
                else:
                    nt = et - 8
                    for dt in range(DT):
                        nc.tensor.matmul(
                            ps[:],
                            lhsT=xt[:, dt, nt * 128 : (nt + 1) * 128],
                            rhs=wq_sb[:, dt, 2 * D : 3 * D],
                            start=(dt == 0),
                            stop=(dt == DT - 1),
                        )
                    nc.vector.tensor_copy(
                        vaug[:, nt, :, 0:DH],
                        ps.rearrange("p (h d) -> p h d", h=HEADS),
                    )

            def norm_chain(s, t, E):
                """Pair t's sums row (E[64], bf16 SBUF) -> recip -> rec2.
                Two SBUF->SBUF DMAs do the partition reshapes; the [32,32]
                layout keeps the (expensive per-element) DVE reciprocal on
                many lanes."""
                rec2 = seq_rec2[s]
                batch = spool.tile([32, 32], BF16, tag="batch")
                nc.sync.dma_start(batch[:], E[64:65, :])
                rec = spool.tile([32, 32], BF16, tag="rec")
                with nc.allow_low_precision(reason="softmax recip bf16"):
                    nc.vector.reciprocal(rec[:], batch[:])
                nc.sync.dma_start(rec2[:, t : t + 1, :], rec[:])

            def r_mult(s, t):
                """Broadcast 1/sums over partitions via P2 matmul, scale oT."""
                oT = seq_oT[s]
                rec2 = seq_rec2[s]
                Rp = psq.tile([128, 512], F32, tag="ps")
                nc.tensor.matmul(
                    Rp[:], lhsT=p2[:], rhs=rec2[:, t, :], start=True, stop=True
                )
                nc.vector.tensor_tensor(oT[:, t, :], oT[:, t, :], Rp[:], MULT)

            def proj_unit(s, dt):
                """One output e-tile of the projection for seq s."""
                oT = seq_oT[s]
                yt = seq_yt[s]
                ps = psq.tile([128, 512], F32, tag="ps")
                for et in range(DT):
                    nc.tensor.matmul(
                        ps[:],
                        lhsT=wo_sb[:, et, dt * 128 : (dt + 1) * 128],
                        rhs=oT[:, et, :],
                        start=(et == 0),
                        stop=(et == DT - 1),
                    )
                nc.vector.tensor_scalar_add(
                    yt[:, dt, :], ps[:], b_sb[:, dt : dt + 1]
                )
                nc.sync.dma_start(out[s, dt], yt[:, dt, :])

            seq_yt = {}

            # ---- prologue: only q[t0]/k[t0] of sequence 0 -----------------
            qkv_alloc(0)  # xt DMAs issue first = startup critical path
            const_dmas()
            qkv_etile(0, 0)
            qkv_etile(0, 4)

            for s in range(SEQ_PER_CORE):
                _, q_sb, k_sb, vaug = seq_tiles[s]
                oT = opool.tile([128, DT, N], BF16, tag="o")
                seq_oT[s] = oT
                seq_rec2[s] = spool.tile([2, 4, N], BF16, tag="rec2", name=f"rec2s{s}")
                if s + 1 < SEQ_PER_CORE:
                    qkv_alloc(s + 1)

                # filler units interleaved into the attention pairs below
                if s == 0:
                    # rest of seq 0's own QKV: v tiles first (attnv needs
                    # them), then the later pairs' q/k tiles just in time
                    own = [
                        (lambda e_=e: qkv_etile(0, e_))
                        for e in (8, 9, 10, 11, 1, 5, 2, 6, 3, 7)
                    ]
                else:
                    own = []
                if s + 1 < SEQ_PER_CORE:
                    qk_units = [
                        (lambda s_=s + 1, e_=e: qkv_etile(s_, e_)) for e in range(12)
                    ]
                else:
                    qk_units = []
                if s >= 1:
                    np_units = [
                        (lambda s_=s - 1, t_=t: r_mult(s_, t_)) for t in range(4)
                    ] + [
                        (lambda s_=s - 1, d_=d: proj_unit(s_, d_)) for d in range(4)
                    ]
                else:
                    np_units = []
                fill = own + qk_units[0:4] + np_units + qk_units[4:12]
                fi = 0
                quota = [0]

                def pair_quota(t):
                    # spread remaining filler units over remaining pairs so
                    # the last pair doesn't starve the PE
                    rem_pairs = 4 - t
                    quota[0] = -(-(len(fill) - fi) // rem_pairs)

                def filler(k):
                    nonlocal fi
                    for _ in range(k):
                        if fi < len(fill) and quota[0] > 0:
                            fill[fi]()
                            fi += 1
                            quota[0] -= 1

                for t in range(4):  # head pair (2t, 2t+1)
                    pair_quota(t)
                    # exp tiles per j-tile: [128, head A|B, 512]
                    expT = [
                        epool.tile(
                            [128, 2, N], BF16, tag=f"expT{jt_}",
                            name=f"expT{jt_}_{s}_{t}",
                        )
                        for jt_ in range(NT)
                    ]
                    od = pso.tile([128, 1024], F32, tag="od")

                    def dots(jh):
                        # d0/d1 hold [A | B] for one j-tile each, so the
                        # row-paired A/B matmuls share a single WAR sem (one
                        # exp op reads the whole tile) and can issue
                        # back-to-back (PE 64-row tile concurrency).
                        dd = [
                            psd.tile(
                                [128, 1024], F32, tag="d0",
                                name=f"d0_{s}_{t}_{jh}",
                            ),
                            psd.tile(
                                [128, 1024], F32, tag="d1",
                                name=f"d1_{s}_{t}_{jh}",
                            ),
                        ]
                        for jj in range(2):
                            # 4-way 64x64 PE tiling: heads A/B on row groups,
                            # j-halves on col groups -> 4 concurrent matmuls
                            jt = 2 * jh + jj
                            c = jt * 128
                            nc.tensor.matmul(
                                dd[jj][0:64, 0:512],
                                lhsT=k_sb[0:64, t, c : c + 64],
                                rhs=q_sb[0:64, t, :],
                                start=True,
                                stop=True,
                            )
                            nc.tensor.matmul(
                                dd[jj][64:128, 0:512],
                                lhsT=k_sb[0:64, t, c + 64 : c + 128],
                                rhs=q_sb[0:64, t, :],
                                start=True,
                                stop=True,
                            )
                            nc.tensor.matmul(
                                dd[jj][0:64, 512:1024],
                                lhsT=k_sb[64:128, t, c : c + 64],
                                rhs=q_sb[64:128, t, :],
                                start=True,
                                stop=True,
                            )
                            nc.tensor.matmul(
                                dd[jj][64:128, 512:1024],
                                lhsT=k_sb[64:128, t, c + 64 : c + 128],
                                rhs=q_sb[64:128, t, :],
                                start=True,
                                stop=True,
                            )
                        for jj in range(2):
                            jt = 2 * jh + jj
                            nc.scalar.activation(
                                expT[jt][:],
                                dd[jj].rearrange("p (a n) -> p a n", a=2),
                                EXP_F,
                                scale=SCALE,
                            )

                    def attnv(jh):
                        for jj in range(2):
                            jt = 2 * jh + jj
                            nc.tensor.matmul(
                                od[0:65, 0:512],
                                lhsT=vaug[:, jt, 2 * t, :],
                                rhs=expT[jt][:, 0, :],
                                start=(jt == 0),
                                stop=(jt == NT - 1),
                            )
                            nc.tensor.matmul(
                                od[0:65, 512:1024],
                                lhsT=vaug[:, jt, 2 * t + 1, :],
                                rhs=expT[jt][:, 1, :],
                                start=(jt == 0),
                                stop=(jt == NT - 1),
                            )

                    dots(0)
                    filler(2)
                    dots(1)
                    filler(2)
                    attnv(0)
                    if s == SEQ_PER_CORE - 1 and t >= 1:
                        # normalize this seq's previous pair in-flight to
                        # shorten the exposed tail (its chain is done by now)
                        r_mult(s, t - 1)
                    else:
                        filler(1)
                    attnv(1)
                    filler(1)

                    # wide evacuation: d-rows AND sums row in one CAST
                    E = Epool.tile([65, 1024], BF16, tag="E")
                    nc.vector.tensor_copy(E[:], od[0:65, :])
                    nc.vector.tensor_copy(oT[0:64, t, :], E[0:64, 0:512])
                    nc.vector.tensor_copy(oT[64:128, t, :], E[0:64, 512:1024])
                    norm_chain(s, t, E)

                seq_yt[s] = ypool.tile([128, DT, N], BF16, tag="y", name=f"yts{s}")
                # drain any unconsumed fillers
                filler(len(fill))

            # ---- tail: last pair's norm + projection of the last sequence -
            s_last = SEQ_PER_CORE - 1
            r_mult(s_last, 3)
            for dt in range(4):
                proj_unit(s_last, dt)

    nc.compile()
    return nc


def make_in_maps(x, W_qkv, W_out, b_out):
    """Shard + lay out full inputs into the 8 per-core input maps."""
    import ml_dtypes

    b, p, n, d = x.shape
    xs = np.ascontiguousarray(x, dtype=np.float32).reshape(b * p, n, d)
    wqkv = (
        np.ascontiguousarray(W_qkv, dtype=np.float32)
        .reshape(DT, 128, 3 * D)
        .astype(ml_dtypes.bfloat16)
    )
    wout = (
        np.ascontiguousarray(W_out, dtype=np.float32)
        .reshape(DT, 128, D)
        .astype(ml_dtypes.bfloat16)
    )
    bo = np.ascontiguousarray(b_out, dtype=np.float32)

    in_maps = []
    for c in range(N_CORES):
        seqs = xs[c * SEQ_PER_CORE : (c + 1) * SEQ_PER_CORE]  # (4, n, d)
        xT = (
            np.ascontiguousarray(seqs.transpose(0, 2, 1))
            .reshape(SEQ_PER_CORE, DT, 128, N)
            .astype(ml_dtypes.bfloat16)
        )
        p2 = np.zeros((2, 128), dtype=ml_dtypes.bfloat16)
        p2[0, 0:64] = 1.0
        p2[1, 64:128] = 1.0
        in_maps.append(
            {"xT": xT, "wqkv": wqkv, "wout": wout, "bout": bo, "p2d": p2}
        )
    return in_maps


def assemble_output(results, b, p, n, d):
    """Gather per-core yT outputs back into the full (b,p,n,d) array."""
    y = np.empty((b * p, n, d), dtype=np.float32)
    for c in range(N_CORES):
        yT = np.asarray(results[c]["out"]).astype(np.float32)
        yT = yT.reshape(SEQ_PER_CORE, D, N)
        y[c * SEQ_PER_CORE : (c + 1) * SEQ_PER_CORE] = yT.transpose(0, 2, 1)
    return y.reshape(b, p, n, d)


_NC_CACHE = None


def _get_nc():
    global _NC_CACHE
    if _NC_CACHE is None:
        _NC_CACHE = build_nc()
    return _NC_CACHE


def run(inputs, trace=False, **spmd_kwargs):
    """Run on the 8 NeuronCores; returns (full_output, BassKernelResults)."""
    from concourse.bass_utils import run_bass_kernel_spmd

    x = np.asarray(inputs["x"])
    b, p, n, d = x.shape
    nc = _get_nc()
    in_maps = make_in_maps(x, inputs["W_qkv"], inputs["W_out"], inputs["b_out"])
    res = run_bass_kernel_spmd(
        nc, in_maps, core_ids=list(range(N_CORES)), trace=trace, **spmd_kwargs
    )
    return assemble_output(res.results, b, p, n, d), res


def kernel(x, W_qkv, W_out, b_out):
    out, _ = run({"x": x, "W_qkv": W_qkv, "W_out": W_out, "b_out": b_out})
    return out.astype(np.float32)


# revision 33
# speedup vs baseline: 1.2319x; 1.2319x over previous
"""Multi-head attention (b=2, p=16, n=512, d=512, h=8, dh=64) on 8 TRN2 cores.

Data-parallel over the 32 (b,p) sequences: 4 sequences per core, no
collectives.  Per-core dataflow (everything "T" = feature-on-partition):

  xT  (d,n)  --W_qkv stationary-->  qT,kT (e,n)   [e-tile = 2 heads]
  xT chunks stationary, W_v moving ->  v natural (n,e)  -> vaug (j,h,65)
  dotsT[j,i] = kT_h.T-slice @ qT_h   (K=64, heads A/B at rows 0:64/64:128)
  expT = exp(scale * dotsT)          (ScalarE, PSUM->SBUF, bf16 out)
  od[0:65] (+ sums row 64) = vaug_h.T @ expT_h   (M=65, ones col -> sums)
  evac: ONE wide CAST od[0:65] -> E (bf16), cheap 4x-mode SBUF re-copies
    E[0:64] -> oT halves; sums row E[64] is SBUF so DMA-able.
  softmax denom: sums -> DRAM -> [64,32] batch -> reciprocal (DVE) ->
    DRAM pair-major reshuffle -> R = P2.T @ rec_pair (PE broadcast) ->
    oT *= R (DVE, in place)
  yT = W_out.T @ oT + b              (bias on ScalarE, bf16 out, per-dt DMA)

Cross-sequence software pipeline: QKV(s+1) + norm/proj(s-1) are
interleaved as filler into the attention pairs of sequence s so the
TensorEngine never starves.  Host transposes x into xT and casts the
bf16 yT output back to f32 y.
"""

import os
import sys

import numpy as np

for _p in ("/opt/trn_rl_repo", "/root/.axon_site/_ro/trn_rl_repo"):
    if os.path.isdir(_p) and _p not in sys.path:
        sys.path.insert(0, _p)

import concourse.bass as bass  # noqa: E402
import concourse.mybir as mybir  # noqa: E402
from concourse import bacc  # noqa: E402
from concourse.tile import TileContext  # noqa: E402

F32 = mybir.dt.float32
BF16 = mybir.dt.bfloat16

N_CORES = 8
SEQ_PER_CORE = 4  # (b*p)=32 sequences / 8 cores
N = 512  # tokens per sequence
D = 512  # model dim
HEADS = 8
DH = 64
SCALE = DH**-0.5
NT = N // 128  # 4 token tiles
DT = D // 128  # 4 dim tiles

EXP_F = mybir.ActivationFunctionType.Exp
COPY_F = mybir.ActivationFunctionType.Copy
MULT = mybir.AluOpType.mult


def build_nc():
    """Build the per-core SPMD Bass program (same program on all 8 cores)."""
    nc = bacc.Bacc("TRN2", target_bir_lowering=False)

    xT = nc.declare_dram_parameter(
        "xT", [SEQ_PER_CORE, DT, 128, N], BF16, isOutput=False
    )
    wqkv = nc.declare_dram_parameter("wqkv", [DT, 128, 3 * D], BF16, isOutput=False)
    wout = nc.declare_dram_parameter("wout", [DT, 128, D], BF16, isOutput=False)
    bout = nc.declare_dram_parameter("bout", [D], F32, isOutput=False)
    p2d = nc.declare_dram_parameter("p2d", [2, 128], BF16, isOutput=False)
    out = nc.declare_dram_parameter(
        "out", [SEQ_PER_CORE, DT, 128, N], BF16, isOutput=True
    )

    with TileContext(nc) as tc:
        with (
            tc.tile_pool(name="consts", bufs=1) as cpool,
            tc.tile_pool(name="xin", bufs=2) as xpool,
            tc.tile_pool(name="qk", bufs=2) as qkpool,
            tc.tile_pool(name="vaug", bufs=2) as vpool,
            tc.tile_pool(name="expt", bufs=3) as epool,
            tc.tile_pool(name="ot", bufs=2) as opool,
            tc.tile_pool(name="evac", bufs=2) as Epool,
            tc.tile_pool(name="small", bufs=2) as spool,
            tc.tile_pool(name="yout", bufs=2) as ypool,
            tc.tile_pool(name="psq", bufs=2, space="PSUM") as psq,
            tc.tile_pool(name="psd", bufs=1, space="PSUM") as psd,
            tc.tile_pool(name="pso", bufs=1, space="PSUM") as pso,
        ):
            # ---- constants: DMA issue order = startup critical path -------
            # (seq-0 x first, then q/k weight halves, then v/out weights)
            wq_sb = cpool.tile([128, DT, 3 * D], BF16, tag="wq")
            wo_sb = cpool.tile([128, DT, D], BF16, tag="wo")

            def const_dmas():
                # q weights, then k, then v; W_out last (tail-only use)
                for lo in (0, 512, 1024):
                    for dt in range(DT):
                        nc.sync.dma_start(
                            wq_sb[:, dt, lo : lo + 512],
                            wqkv[dt][:, lo : lo + 512],
                        )
                for dt in range(DT):
                    nc.sync.dma_start(wo_sb[:, dt, :], wout[dt])

            b_sb = cpool.tile([128, DT], F32, tag="b")
            nc.sync.dma_start(b_sb[:], bout.rearrange("(t p) -> p t", p=128))
            p2 = cpool.tile([2, 128], BF16, tag="p2")
            nc.sync.dma_start(p2[:], p2d[:])

            seq_tiles = {}
            seq_oT = {}
            seq_rec2 = {}
            seq_scr = {}

            def qkv_alloc(s):
                """Allocate per-seq tiles + start the xT DMAs (one per dt)."""
                xt = xpool.tile([128, DT, N], BF16, tag="x")
                for dt in range(DT):
                    nc.sync.dma_start(xt[:, dt, :], xT[s, dt])
                q_sb = qkpool.tile([128, DT, N], BF16, tag="q")
                k_sb = qkpool.tile([128, DT, N], BF16, tag="k")
                vaug = vpool.tile([128, NT, HEADS, DH + 1], BF16, tag="v")
                nc.vector.memset(vaug[:, :, :, DH : DH + 1], 1.0)
                seq_tiles[s] = (xt, q_sb, k_sb, vaug)

            def qkv_etile(s, et):
                """One QKV output tile (et 0..7 = qT/kT e-tiles, 8..11 = v
                natural n-tiles): 4 accumulating matmuls + evacuation."""
                xt, q_sb, k_sb, vaug = seq_tiles[s]
                ps = psq.tile([128, 512], F32, tag="ps")
                if et < 8:
                    for dt in range(DT):
                        nc.tensor.matmul(
                            ps[:],
                            lhsT=wq_sb[:, dt, et * 128 : (et + 1) * 128],
                            rhs=xt[:, dt, :],
                            start=(dt == 0),
                            stop=(dt == DT - 1),
                        )
                    if et < 4:
                        # q evac on ScalarE to balance engine load
                        nc.scalar.activation(q_sb[:, et, :], ps[:], COPY_F)
                    else:
                        nc.vector.tensor_copy(k_sb[:, et - 4, :], ps[:])
                else:
                    nt = et - 8
                    for dt in range(DT):
                        nc.tensor.matmul(
                            ps[:],
                            lhsT=xt[:, dt, nt * 128 : (nt + 1) * 128],
                            rhs=wq_sb[:, dt, 2 * D : 3 * D],
                            start=(dt == 0),
                            stop=(dt == DT - 1),
                        )
                    nc.vector.tensor_copy(
                        vaug[:, nt, :, 0:DH],
                        ps.rearrange("p (h d) -> p h d", h=HEADS),
                    )

            def norm_chain(s, t, E):
                """Pair t's sums row (E[64], bf16 SBUF) -> recip -> rec2.
                Two SBUF->SBUF DMAs do the partition reshapes; the [32,32]
                layout keeps the (expensive per-element) DVE reciprocal on
                many lanes."""
                rec2 = seq_rec2[s]
                batch = spool.tile([32, 32], BF16, tag="batch")
                nc.sync.dma_start(batch[:], E[64:65, :])
                rec = spool.tile([32, 32], BF16, tag="rec")
                with nc.allow_low_precision(reason="softmax recip bf16"):
                    nc.vector.reciprocal(rec[:], batch[:])
                nc.sync.dma_start(rec2[:, t : t + 1, :], rec[:])

            def r_mult(s, t):
                """Broadcast 1/sums over partitions via P2 matmul, scale oT."""
                oT = seq_oT[s]
                rec2 = seq_rec2[s]
                Rp = psq.tile([128, 512], F32, tag="ps")
                nc.tensor.matmul(
                    Rp[:], lhsT=p2[:], rhs=rec2[:, t, :], start=True, stop=True
                )
                nc.vector.tensor_tensor(oT[:, t, :], oT[:, t, :], Rp[:], MULT)

            def proj_unit(s, dt):
                """One output e-tile of the projection for seq s."""
                oT = seq_oT[s]
                yt = seq_yt[s]
                ps = psq.tile([128, 512], F32, tag="ps")
                for et in range(DT):
                    nc.tensor.matmul(
                        ps[:],
                        lhsT=wo_sb[:, et, dt * 128 : (dt + 1) * 128],
                        rhs=oT[:, et, :],
                        start=(et == 0),
                        stop=(et == DT - 1),
                    )
                nc.vector.tensor_scalar_add(
                    yt[:, dt, :], ps[:], b_sb[:, dt : dt + 1]
                )
                nc.sync.dma_start(out[s, dt], yt[:, dt, :])

            seq_yt = {}

            # ---- prologue: only q[t0]/k[t0] of sequence 0 -----------------
            qkv_alloc(0)  # xt DMAs issue first = startup critical path
            const_dmas()
            qkv_etile(0, 0)
            qkv_etile(0, 4)

            for s in range(SEQ_PER_CORE):
                _, q_sb, k_sb, vaug = seq_tiles[s]
                oT = opool.tile([128, DT, N], BF16, tag="o")
                seq_oT[s] = oT
                seq_rec2[s] = spool.tile([2, 4, N], BF16, tag="rec2", name=f"rec2s{s}")
                if s + 1 < SEQ_PER_CORE:
                    qkv_alloc(s + 1)

                # filler units interleaved into the attention pairs below
                if s == 0:
                    # rest of seq 0's own QKV: v tiles first (attnv needs
                    # them), then the later pairs' q/k tiles just in time
                    own = [
                        (lambda e_=e: qkv_etile(0, e_))
                        for e in (8, 9, 10, 11, 1, 5, 2, 6, 3, 7)
                    ]
                else:
                    own = []
                if s + 1 < SEQ_PER_CORE:
                    qk_units = [
                        (lambda s_=s + 1, e_=e: qkv_etile(s_, e_)) for e in range(12)
                    ]
                else:
                    qk_units = []
                if s >= 1:
                    np_units = [
                        (lambda s_=s - 1, t_=t: r_mult(s_, t_)) for t in range(4)
                    ] + [
                        (lambda s_=s - 1, d_=d: proj_unit(s_, d_)) for d in range(4)
                    ]
                else:
                    np_units = []
                fill = own + qk_units[0:4] + np_units + qk_units[4:12]
                n_must = len(own)  # seq-0's own q/k/v tiles: hard deps of
                fi = 0             # the very next pairs, quota-exempt
                quota = [0]

                def pair_quota(t):
                    # spread remaining filler units over remaining pairs,
                    # reserving a share for the seq-boundary drain so the
                    # first dots of the next sequence doesn't starve
                    rem_pairs = 5 - t
                    quota[0] = -(-(len(fill) - fi) // rem_pairs)

                def filler(k):
                    nonlocal fi
                    for _ in range(k):
                        if fi >= len(fill):
                            return
                        if fi < n_must:
                            fill[fi]()
                            fi += 1
                        elif quota[0] > 0:
                            fill[fi]()
                            fi += 1
                            quota[0] -= 1

                for t in range(4):  # head pair (2t, 2t+1)
                    pair_quota(t)
                    # exp tiles per j-tile: [128, head A|B, 512]
                    expT = [
                        epool.tile(
                            [128, 2, N], BF16, tag=f"expT{jt_}",
                            name=f"expT{jt_}_{s}_{t}",
                        )
                        for jt_ in range(NT)
                    ]
                    od = pso.tile([128, 1024], F32, tag="od")

                    def dots(jh):
                        # d0/d1 hold [A | B] for one j-tile each, so the
                        # row-paired A/B matmuls share a single WAR sem (one
                        # exp op reads the whole tile) and can issue
                        # back-to-back (PE 64-row tile concurrency).
                        dd = [
                            psd.tile(
                                [128, 1024], F32, tag="d0",
                                name=f"d0_{s}_{t}_{jh}",
                            ),
                            psd.tile(
                                [128, 1024], F32, tag="d1",
                                name=f"d1_{s}_{t}_{jh}",
                            ),
                        ]
                        for jj in range(2):
                            jt = 2 * jh + jj
                            nc.tensor.matmul(
                                dd[jj][:, 0:512],
                                lhsT=k_sb[0:64, t, jt * 128 : (jt + 1) * 128],
                                rhs=q_sb[0:64, t, :],
                                start=True,
                                stop=True,
                            )
                            nc.tensor.matmul(
                                dd[jj][:, 512:1024],
                                lhsT=k_sb[64:128, t, jt * 128 : (jt + 1) * 128],
                                rhs=q_sb[64:128, t, :],
                                start=True,
                                stop=True,
                            )
                        for jj in range(2):
                            jt = 2 * jh + jj
                            nc.scalar.activation(
                                expT[jt][:],
                                dd[jj].rearrange("p (a n) -> p a n", a=2),
                                EXP_F,
                                scale=SCALE,
                            )

                    def attnv(jh):
                        for jj in range(2):
                            jt = 2 * jh + jj
                            nc.tensor.matmul(
                                od[0:65, 0:512],
                                lhsT=vaug[:, jt, 2 * t, :],
                                rhs=expT[jt][:, 0, :],
                                start=(jt == 0),
                                stop=(jt == NT - 1),
                            )
                            nc.tensor.matmul(
                                od[0:65, 512:1024],
                                lhsT=vaug[:, jt, 2 * t + 1, :],
                                rhs=expT[jt][:, 1, :],
                                start=(jt == 0),
                                stop=(jt == NT - 1),
                            )

                    dots(0)
                    filler(2)
                    dots(1)
                    filler(2)
                    attnv(0)
                    if s == SEQ_PER_CORE - 1 and t >= 1:
                        # normalize this seq's previous pair in-flight to
                        # shorten the exposed tail (its chain is done by now)
                        r_mult(s, t - 1)
                    else:
                        filler(1)
                    attnv(1)
                    filler(1)

                    # wide evacuation: d-rows AND sums row in one CAST
                    E = Epool.tile([65, 1024], BF16, tag="E")
                    nc.vector.tensor_copy(E[:], od[0:65, :])
                    nc.vector.tensor_copy(oT[0:64, t, :], E[0:64, 0:512])
                    nc.vector.tensor_copy(oT[64:128, t, :], E[0:64, 512:1024])
                    norm_chain(s, t, E)

                seq_yt[s] = ypool.tile([128, DT, N], BF16, tag="y", name=f"yts{s}")
                # drain any unconsumed fillers (quota-exempt)
                quota[0] = len(fill)
                filler(len(fill))

            # ---- tail: last pair's norm + projection of the last sequence -
            s_last = SEQ_PER_CORE - 1
            r_mult(s_last, 3)
            for dt in range(4):
                proj_unit(s_last, dt)

    nc.compile()
    return nc


def make_in_maps(x, W_qkv, W_out, b_out):
    """Shard + lay out full inputs into the 8 per-core input maps."""
    import ml_dtypes

    b, p, n, d = x.shape
    xs = np.ascontiguousarray(x, dtype=np.float32).reshape(b * p, n, d)
    wqkv = (
        np.ascontiguousarray(W_qkv, dtype=np.float32)
        .reshape(DT, 128, 3 * D)
        .astype(ml_dtypes.bfloat16)
    )
    wout = (
        np.ascontiguousarray(W_out, dtype=np.float32)
        .reshape(DT, 128, D)
        .astype(ml_dtypes.bfloat16)
    )
    bo = np.ascontiguousarray(b_out, dtype=np.float32)

    in_maps = []
    for c in range(N_CORES):
        seqs = xs[c * SEQ_PER_CORE : (c + 1) * SEQ_PER_CORE]  # (4, n, d)
        xT = (
            np.ascontiguousarray(seqs.transpose(0, 2, 1))
            .reshape(SEQ_PER_CORE, DT, 128, N)
            .astype(ml_dtypes.bfloat16)
        )
        p2 = np.zeros((2, 128), dtype=ml_dtypes.bfloat16)
        p2[0, 0:64] = 1.0
        p2[1, 64:128] = 1.0
        in_maps.append(
            {"xT": xT, "wqkv": wqkv, "wout": wout, "bout": bo, "p2d": p2}
        )
    return in_maps


def assemble_output(results, b, p, n, d):
    """Gather per-core yT outputs back into the full (b,p,n,d) array."""
    y = np.empty((b * p, n, d), dtype=np.float32)
    for c in range(N_CORES):
        yT = np.asarray(results[c]["out"]).astype(np.float32)
        yT = yT.reshape(SEQ_PER_CORE, D, N)
        y[c * SEQ_PER_CORE : (c + 1) * SEQ_PER_CORE] = yT.transpose(0, 2, 1)
    return y.reshape(b, p, n, d)


_NC_CACHE = None


def _get_nc():
    global _NC_CACHE
    if _NC_CACHE is None:
        _NC_CACHE = build_nc()
    return _NC_CACHE


def run(inputs, trace=False, **spmd_kwargs):
    """Run on the 8 NeuronCores; returns (full_output, BassKernelResults)."""
    from concourse.bass_utils import run_bass_kernel_spmd

    x = np.asarray(inputs["x"])
    b, p, n, d = x.shape
    nc = _get_nc()
    in_maps = make_in_maps(x, inputs["W_qkv"], inputs["W_out"], inputs["b_out"])
    res = run_bass_kernel_spmd(
        nc, in_maps, core_ids=list(range(N_CORES)), trace=trace, **spmd_kwargs
    )
    return assemble_output(res.results, b, p, n, d), res


def kernel(x, W_qkv, W_out, b_out):
    out, _ = run({"x": x, "W_qkv": W_qkv, "W_out": W_out, "b_out": b_out})
    return out.astype(np.float32)
